# revision 5
# baseline (speedup 1.0000x reference)
import os
import sys

import ml_dtypes
import numpy as np

if "/opt/trn_rl_repo" not in sys.path:
    sys.path.insert(0, "/opt/trn_rl_repo")

import concourse.bass as bass
import concourse.mybir as mybir
import concourse.tile as tile
from concourse import bacc, bass_utils
from concourse.bass import ds, ts

B, C, W, H, D = 4, 512, 2048, 4, 64
P = 128
CT = C // P  # 4 contraction tiles of 128 over channels
IT = W // P  # 16 row blocks over sequence
JT = W // 512  # 4 column chunks of 512 over sequence
ET = C // P  # 4 output-channel blocks
FP32 = mybir.dt.float32
BF16 = mybir.dt.bfloat16
F8 = mybir.dt.float8e4
E4M3 = ml_dtypes.float8_e4m3

# fp8 scaling bookkeeping:
#   wq8 = 32*(Wq^T/sqrt(D)), wk8 = 32*Wk^T  -> scores s' = 1024*s
#   exp: p = exp(s'/1024 - ln 8) = e^s/8  (keeps e4m3 in normal range)
#   rsum_raw = sum_j p = S/8; rinv = 8/S
#   wv8 = 128*Wv^T -> vp = 128*v; vt8 = vp*rinv = 1024*v/S
#   ctx' = vt8 @ p = 128*ctx; residual rs = 256*x on even cores; host /128
QK_SCALE = 32.0
V_SCALE = 128.0
GAMMA = 128.0
ACT_SCALE = 1.0 / (QK_SCALE * QK_SCALE)
EXP_BIAS = -2.0794415416798357  # -ln(8)

_NC_CACHE = None
LAST_EXEC_NS = None
LAST_MEAN_EXEC_NS = None


def _build():
    nc = bacc.Bacc("TRN2", target_bir_lowering=False)
    x8_d = nc.dram_tensor("x8", (C, W), F8, kind="ExternalInput")
    xb_d = nc.dram_tensor("xb", (C, W), BF16, kind="ExternalInput")
    wq_d = nc.dram_tensor("wq", (2, C, D), F8, kind="ExternalInput")
    wk_d = nc.dram_tensor("wk", (2, C, D), F8, kind="ExternalInput")
    wv_d = nc.dram_tensor("wv", (2, C, C), F8, kind="ExternalInput")
    rs_d = nc.dram_tensor("rs", (P, 1), FP32, kind="ExternalInput")
    out_d = nc.dram_tensor("out", (C, W), FP32, kind="ExternalOutput")

    with tile.TileContext(nc) as tc:
        with (
            tc.tile_pool(name="sb", bufs=1) as sb,
            tc.tile_pool(name="ps", bufs=1, space="PSUM") as ps,
        ):
            x8_sb = sb.tile((P, CT, W), F8)
            xb_sb = sb.tile((P, CT, W), BF16)
            wq_sb = sb.tile((P, 2, CT, D), F8)
            wk_sb = sb.tile((P, 2, CT, D), F8)
            wv_sb = sb.tile((P, 2, CT, C), F8)
            rs_sb = sb.tile((P, 1), FP32)
            eb_sb = sb.tile((P, 1), FP32)
            warm_sb = sb.tile((P, 512), BF16)
            outa = sb.tile((P, ET, W), FP32)
            q1_sb = sb.tile((D, W), BF16)
            k1_sb = sb.tile((D, W), BF16)
            q2_sb = sb.tile((D, W), BF16)
            k2_sb = sb.tile((D, W), BF16)
            v1_sb = sb.tile((P, IT, C), BF16)  # head1 v^T staging (bf16)
            p_sb = sb.tile((P, 2, IT, JT, 512), F8)
            vt8_sb = sb.tile((P, 2, IT, C), F8)
            sums = sb.tile((P, 2, IT, 2), FP32)  # exp accums per (h, it, j2)
            rsum = sb.tile((P, 2, IT), FP32)
            rinv = sb.tile((P, 2, IT), FP32)

            qs = [nc.sync, nc.gpsimd, nc.scalar]
            # batched input DMAs, spread over queues; x8+wq/wk gate compute
            nc.gpsimd.memset(warm_sb[:], 0.0)
            nc.gpsimd.memset(eb_sb[:], EXP_BIAS)
            nc.gpsimd.dma_start(rs_sb[:], rs_d[:])
            nc.sync.dma_start(
                wq_sb[:], wq_d[:].rearrange("h (ct p) d -> p h ct d", p=P)
            )
            nc.sync.dma_start(
                wk_sb[:], wk_d[:].rearrange("h (ct p) d -> p h ct d", p=P)
            )
            for ct in range(CT):
                [nc.sync, nc.scalar][ct % 2].dma_start(
                    x8_sb[:, ct, 0:512], x8_d[ts(ct, P), 0:512]
                )
            for ct in range(CT):
                [nc.sync, nc.scalar][ct % 2].dma_start(
                    x8_sb[:, ct, 512:W], x8_d[ts(ct, P), 512:W]
                )
            nc.gpsimd.dma_start(
                wv_sb[:], wv_d[:].rearrange("h (ct p) e -> p h ct e", p=P)
            )
            xb_src = xb_d[:].rearrange("(ct p) w -> p ct w", p=P)
            for half in range(2):
                nc.gpsimd.dma_start(
                    xb_sb[:, ds(2 * half, 2)], xb_src[:, ds(2 * half, 2)]
                )

            DR = mybir.MatmulPerfMode.DoubleRow

            # HAM warm-up: dummy matmuls so the PE clock is at 8/8 when real
            # work (gated on DMA arrival) begins
            wp = ps.tile((P, 512), FP32, tag="gp", bufs=4, name="wp")
            for _ in range(6):
                nc.tensor.matmul(wp[:], warm_sb[:, 0:128], warm_sb[:])

            def qk_nt(h, nt, qd, kd):
                qp = ps.tile((P, 512), FP32, tag="gp", bufs=4, name="qp")
                kp = ps.tile((P, 512), FP32, tag="gp", bufs=4, name="kp")
                for cc in range(CT // 2):
                    nc.tensor.matmul(
                        qp[0:D, :],
                        wq_sb[:, h, ds(2 * cc, 2), :],
                        x8_sb[:, ds(2 * cc, 2), ts(nt, 512)],
                        start=(cc == 0),
                        stop=(cc == CT // 2 - 1),
                        perf_mode=DR,
                    )
                for cc in range(CT // 2):
                    nc.tensor.matmul(
                        kp[0:D, :],
                        wk_sb[:, h, ds(2 * cc, 2), :],
                        x8_sb[:, ds(2 * cc, 2), ts(nt, 512)],
                        start=(cc == 0),
                        stop=(cc == CT // 2 - 1),
                        perf_mode=DR,
                    )
                if h == 0:
                    nc.scalar.copy(qd[:, ts(nt, 512)], qp[0:D, :])
                else:
                    nc.vector.tensor_copy(qd[:, ts(nt, 512)], qp[0:D, :])
                nc.vector.tensor_copy(kd[:, ts(nt, 512)], kp[0:D, :])

            def sc_exp(h, it, qd, kd):
                # row-sums ride the ACT accumulator (free during ACTIVATE)
                for j2 in range(2):
                    sp = ps.tile((P, 2, 512), FP32, tag="sc", bufs=2, name="sp")
                    for jh in range(2):
                        nc.tensor.matmul(
                            sp[:, jh],
                            qd[:, ts(it, P)],
                            kd[:, ds(j2 * 1024 + jh * 512, 512)],
                        )
                    nc.scalar.activation(
                        p_sb[:, h, it, ds(2 * j2, 2)],
                        sp[:],
                        mybir.ActivationFunctionType.Exp,
                        bias=eb_sb[:],
                        scale=ACT_SCALE,
                        accum_out=sums[:, h, it, ds(j2, 1)],
                    )

            def vt_mm(h, it):
                vp = ps.tile((P, 512), FP32, tag="gp", bufs=4, name="vp")
                for cc in range(CT // 2):
                    nc.tensor.matmul(
                        vp[:],
                        x8_sb[:, ds(2 * cc, 2), ts(it, P)],
                        wv_sb[:, h, ds(2 * cc, 2), :],
                        start=(cc == 0),
                        stop=(cc == CT // 2 - 1),
                        perf_mode=DR,
                    )
                return vp

            def rinv_it(h, it):
                nc.vector.tensor_add(
                    rsum[:, h, ds(it, 1)],
                    sums[:, h, it, ds(0, 1)],
                    sums[:, h, it, ds(1, 1)],
                )
                nc.vector.reciprocal(rinv[:, h, ds(it, 1)], rsum[:, h, ds(it, 1)])

            def ctx_chunk(h, et, jt, dma_out):
                cp = ps.tile((P, 512), FP32, tag="gp", bufs=4, name="cp")
                for kk in range(IT // 2):
                    nc.tensor.matmul(
                        cp[:],
                        vt8_sb[:, h, ds(2 * kk, 2), ts(et, P)],
                        p_sb[:, h, ds(2 * kk, 2), jt],
                        start=(kk == 0),
                        stop=(kk == IT // 2 - 1),
                        perf_mode=DR,
                    )
                nc.vector.tensor_add(
                    outa[:, et, ts(jt, 512)], outa[:, et, ts(jt, 512)], cp[:]
                )
                if dma_out:
                    eng = qs[(et * JT + jt) % 3]
                    eng.dma_start(
                        out_d[ts(et, P), ts(jt, 512)], outa[:, et, ts(jt, 512)]
                    )

            # phase 0: h0 qk projection only (k nt0/nt1 first for exp(0) gate)
            for nt in (0, 1, 2, 3):
                qk_nt(0, nt, q1_sb, k1_sb)

            # phase 1: ACT-bound exp h0; PE also does v^T h1 + h1 qk proj
            for it in range(IT):
                sc_exp(0, it, q1_sb, k1_sb)
                vp0 = vt_mm(0, it)
                rinv_it(0, it)
                nc.vector.tensor_scalar_mul(
                    vt8_sb[:, 0, it], vp0[:], rinv[:, 0, ds(it, 1)]
                )
                if it < JT:
                    qk_nt(1, it, q2_sb, k2_sb)
                vp1 = vt_mm(1, it)
                nc.vector.tensor_copy(v1_sb[:, it], vp1[:])
                if it == 14:
                    nc.vector.tensor_scalar_mul(outa[:, 0], xb_sb[:, 0], rs_sb[:])

            # phase 2: ACT exp h1; PE ctx h0 lagged one it behind the scores
            for it in range(IT):
                sc_exp(1, it, q2_sb, k2_sb)
                rinv_it(1, it)
                nc.vector.tensor_scalar_mul(
                    vt8_sb[:, 1, it], v1_sb[:, it], rinv[:, 1, ds(it, 1)]
                )
                if it in (2, 6, 10):
                    ct = it // 4 + 1
                    nc.vector.tensor_scalar_mul(outa[:, ct], xb_sb[:, ct], rs_sb[:])
                if it >= 1:
                    t = it - 1
                    ctx_chunk(0, t // JT, t % JT, dma_out=False)
            ctx_chunk(0, 3, 3, dma_out=False)

            # phase 3: ctx h1, ACT idle
            for et in range(ET):
                for jt in range(JT):
                    ctx_chunk(1, et, jt, dma_out=True)

    nc.finalize()
    return nc


def kernel(x, Wq, bq, Wk, bk, Wv, bv):
    global _NC_CACHE, LAST_EXEC_NS, LAST_MEAN_EXEC_NS
    x = np.ascontiguousarray(np.asarray(x, dtype=np.float32))
    Wq = np.asarray(Wq, dtype=np.float32)
    Wk = np.asarray(Wk, dtype=np.float32)
    Wv = np.asarray(Wv, dtype=np.float32)
    scale = np.float32(D ** -0.5)

    if _NC_CACHE is None:
        _NC_CACHE = _build()
    nc = _NC_CACHE

    x8 = x.astype(E4M3)
    xb = x.astype(ml_dtypes.bfloat16)

    # core c -> batch c//2, head pair c%2 (heads 2p, 2p+1)
    wq_pair = []
    wk_pair = []
    wv_pair = []
    for pair in range(2):
        hs = [2 * pair, 2 * pair + 1]
        wq_pair.append(
            np.ascontiguousarray(
                (np.stack([Wq[h].T for h in hs]) * (QK_SCALE * scale)).astype(E4M3)
            )
        )
        wk_pair.append(
            np.ascontiguousarray(
                (np.stack([Wk[h].T for h in hs]) * QK_SCALE).astype(E4M3)
            )
        )
        wv_pair.append(
            np.ascontiguousarray(
                (np.stack([Wv[h].T for h in hs]) * V_SCALE).astype(E4M3)
            )
        )

    in_maps = []
    for c in range(8):
        b, pair = c // 2, c % 2
        in_maps.append(
            {
                "x8": x8[b],
                "xb": xb[b],
                "wq": wq_pair[pair],
                "wk": wk_pair[pair],
                "wv": wv_pair[pair],
                "rs": np.full(
                    (P, 1), 2.0 * GAMMA if pair == 0 else 0.0, dtype=np.float32
                ),
            }
        )

    res = bass_utils.run_bass_kernel_spmd(nc, in_maps, core_ids=list(range(8)))
    LAST_EXEC_NS = res.exec_time_ns
    LAST_MEAN_EXEC_NS = res.mean_exec_time_ns

    out = np.empty((B, C, W), dtype=np.float32)
    inv_g = np.float32(1.0 / GAMMA)
    for b in range(B):
        out[b] = (res.results[2 * b]["out"] + res.results[2 * b + 1]["out"]) * inv_g
    return out


# revision 6
# speedup vs baseline: 1.0830x; 1.0830x over previous
import os
import sys

import ml_dtypes
import numpy as np

if "/opt/trn_rl_repo" not in sys.path:
    sys.path.insert(0, "/opt/trn_rl_repo")

import concourse.bass as bass
import concourse.mybir as mybir
import concourse.tile as tile
from concourse import bacc, bass_utils
from concourse.bass import ds, ts

B, C, W, H, D = 4, 512, 2048, 4, 64
P = 128
CT = C // P  # 4 contraction tiles of 128 over channels
IT = W // P  # 16 row blocks over sequence
JT = W // 512  # 4 column chunks of 512 over sequence
ET = C // P  # 4 output-channel blocks
FP32 = mybir.dt.float32
BF16 = mybir.dt.bfloat16
F8 = mybir.dt.float8e4
E4M3 = ml_dtypes.float8_e4m3

# fp8 scaling bookkeeping:
#   wqs = 32*(Wq^T/sqrt(D)) both heads stacked -> scores s' = 1024*s
#   exp: p = exp(s'/1024 - ln 8) = e^s/8  (keeps e4m3 in normal range)
#   rsum_raw = sum_j p = S/8; rinv = 8/S
#   wv8 = 128*Wv^T -> vp = 128*v; vt8 = vp*rinv = 1024*v/S
#   ctx' = vt8 @ p = 128*ctx; residual rs = 256*x on even cores; host /128
QK_SCALE = 32.0
V_SCALE = 128.0
GAMMA = 128.0
ACT_SCALE = 1.0 / (QK_SCALE * QK_SCALE)
EXP_BIAS = -2.0794415416798357  # -ln(8)

_NC_CACHE = None
LAST_EXEC_NS = None
LAST_MEAN_EXEC_NS = None


def _build():
    nc = bacc.Bacc("TRN2", target_bir_lowering=False)
    x8_d = nc.dram_tensor("x8", (C, W), F8, kind="ExternalInput")
    xb_d = nc.dram_tensor("xb", (C, W), BF16, kind="ExternalInput")
    wqs_d = nc.dram_tensor("wqs", (C, P), F8, kind="ExternalInput")
    wks_d = nc.dram_tensor("wks", (C, P), F8, kind="ExternalInput")
    wv_d = nc.dram_tensor("wv", (2, C, C), F8, kind="ExternalInput")
    rs_d = nc.dram_tensor("rs", (P, 1), FP32, kind="ExternalInput")
    out_d = nc.dram_tensor("out", (C, W), FP32, kind="ExternalOutput")

    with tile.TileContext(nc) as tc:
        with (
            tc.tile_pool(name="sb", bufs=1) as sb,
            tc.tile_pool(name="ps", bufs=1, space="PSUM") as ps,
        ):
            x8_sb = sb.tile((P, CT, W), F8)
            xb_sb = sb.tile((P, CT, W), BF16)
            wqs_sb = sb.tile((P, CT, P), F8)
            wks_sb = sb.tile((P, CT, P), F8)
            wv_sb = sb.tile((P, 2, CT, C), F8)
            rs_sb = sb.tile((P, 1), FP32)
            eb_sb = sb.tile((P, 1), FP32)
            warm_sb = sb.tile((P, 512), BF16)
            outa = sb.tile((P, ET, W), FP32)
            qb_sb = sb.tile((P, W), BF16)  # rows 0-63 h0 q, 64-127 h1 q
            kb_sb = sb.tile((P, W), BF16)
            v1_sb = sb.tile((P, IT, C), BF16)  # head1 v^T staging (bf16)
            p_sb = sb.tile((P, 2, IT, JT, 512), F8)
            vt8_sb = sb.tile((P, 2, IT, C), F8)
            sums = sb.tile((P, 2, IT, 2), FP32)  # exp accums per (h, it, j2)
            rsum = sb.tile((P, 2, IT), FP32)
            rinv = sb.tile((P, 2, IT), FP32)

            qs = [nc.sync, nc.gpsimd, nc.scalar]
            # input DMAs: contiguous chunks spread over all three queues;
            # first-needed first (wqs/wks + x8 low columns gate compute)
            nc.gpsimd.memset(warm_sb[:], 0.0)
            nc.gpsimd.memset(eb_sb[:], EXP_BIAS)
            for ct in range(CT):
                nc.sync.dma_start(wqs_sb[:, ct], wqs_d[ts(ct, P), :])
            for ct in range(CT):
                nc.scalar.dma_start(wks_sb[:, ct], wks_d[ts(ct, P), :])
            for ct in range(CT):
                nc.gpsimd.dma_start(x8_sb[:, ct, 0:512], x8_d[ts(ct, P), 0:512])
            for ct in range(CT):
                [nc.sync, nc.scalar][ct % 2].dma_start(
                    x8_sb[:, ct, 512:W], x8_d[ts(ct, P), 512:W]
                )
            for ct in range(CT):
                nc.sync.dma_start(wv_sb[:, 0, ct], wv_d[0, ts(ct, P), :])
            for ct in range(CT):
                nc.scalar.dma_start(wv_sb[:, 1, ct], wv_d[1, ts(ct, P), :])
            nc.gpsimd.dma_start(rs_sb[:], rs_d[:])
            xb_src = xb_d[:].rearrange("(ct p) w -> p ct w", p=P)
            for half in range(2):
                nc.gpsimd.dma_start(
                    xb_sb[:, ds(2 * half, 2)], xb_src[:, ds(2 * half, 2)]
                )

            DR = mybir.MatmulPerfMode.DoubleRow

            # HAM warm-up: dummy matmuls so the PE clock is at 8/8 when real
            # work (gated on DMA arrival) begins
            wp = ps.tile((P, 512), FP32, tag="gp", bufs=2, name="wp")
            for _ in range(9):
                nc.tensor.matmul(wp[:], warm_sb[:, 0:128], warm_sb[:])

            def qk_nt(nt):
                # both heads' q (or k) in one stacked M=128 chain
                qp = ps.tile((P, 512), FP32, tag="gp", bufs=2, name="qp")
                for cc in range(CT // 2):
                    nc.tensor.matmul(
                        qp[:],
                        wqs_sb[:, ds(2 * cc, 2), :],
                        x8_sb[:, ds(2 * cc, 2), ts(nt, 512)],
                        start=(cc == 0),
                        stop=(cc == CT // 2 - 1),
                        perf_mode=DR,
                    )
                nc.scalar.copy(qb_sb[:, ts(nt, 512)], qp[:])
                kp = ps.tile((P, 512), FP32, tag="gp", bufs=2, name="kp")
                for cc in range(CT // 2):
                    nc.tensor.matmul(
                        kp[:],
                        wks_sb[:, ds(2 * cc, 2), :],
                        x8_sb[:, ds(2 * cc, 2), ts(nt, 512)],
                        start=(cc == 0),
                        stop=(cc == CT // 2 - 1),
                        perf_mode=DR,
                    )
                nc.vector.tensor_copy(kb_sb[:, ts(nt, 512)], kp[:])

            def sc_exp(h, it):
                # h0 in array rows 0-63, h1 in rows 64-127 (row-group MMs)
                lo, hi = (0, D) if h == 0 else (D, P)
                for j2 in range(2):
                    sp = ps.tile((P, 2, 512), FP32, tag="sc", bufs=3, name="sp")
                    for jh in range(2):
                        nc.tensor.matmul(
                            sp[:, jh],
                            qb_sb[lo:hi, ts(it, P)],
                            kb_sb[lo:hi, ds(j2 * 1024 + jh * 512, 512)],
                        )
                    nc.scalar.activation(
                        p_sb[:, h, it, ds(2 * j2, 2)],
                        sp[:],
                        mybir.ActivationFunctionType.Exp,
                        bias=eb_sb[:],
                        scale=ACT_SCALE,
                        accum_out=sums[:, h, it, ds(j2, 1)],
                    )

            def vt_mm(h, it):
                vp = ps.tile((P, 512), FP32, tag="gp", bufs=2, name="vp")
                for cc in range(CT // 2):
                    nc.tensor.matmul(
                        vp[:],
                        x8_sb[:, ds(2 * cc, 2), ts(it, P)],
                        wv_sb[:, h, ds(2 * cc, 2), :],
                        start=(cc == 0),
                        stop=(cc == CT // 2 - 1),
                        perf_mode=DR,
                    )
                return vp

            def rinv_it(h, it):
                nc.vector.tensor_add(
                    rsum[:, h, ds(it, 1)],
                    sums[:, h, it, ds(0, 1)],
                    sums[:, h, it, ds(1, 1)],
                )
                nc.vector.reciprocal(rinv[:, h, ds(it, 1)], rsum[:, h, ds(it, 1)])

            def ctx_chunk(h, et, jt, dma_out):
                cp = ps.tile((P, 512), FP32, tag="gp", bufs=2, name="cp")
                for kk in range(IT // 2):
                    nc.tensor.matmul(
                        cp[:],
                        vt8_sb[:, h, ds(2 * kk, 2), ts(et, P)],
                        p_sb[:, h, ds(2 * kk, 2), jt],
                        start=(kk == 0),
                        stop=(kk == IT // 2 - 1),
                        perf_mode=DR,
                    )
                nc.vector.tensor_add(
                    outa[:, et, ts(jt, 512)], outa[:, et, ts(jt, 512)], cp[:]
                )
                if dma_out:
                    eng = qs[(et * JT + jt) % 3]
                    eng.dma_start(
                        out_d[ts(et, P), ts(jt, 512)], outa[:, et, ts(jt, 512)]
                    )

            # phase 0: stacked qk projection (both heads per chain)
            for nt in range(JT):
                qk_nt(nt)

            # phase 1: ACT-bound exp h0; PE also precomputes v^T h1
            for it in range(IT):
                sc_exp(0, it)
                vp0 = vt_mm(0, it)
                rinv_it(0, it)
                nc.vector.tensor_scalar_mul(
                    vt8_sb[:, 0, it], vp0[:], rinv[:, 0, ds(it, 1)]
                )
                vp1 = vt_mm(1, it)
                nc.vector.tensor_copy(v1_sb[:, it], vp1[:])
                if it == 14:
                    nc.vector.tensor_scalar_mul(outa[:, 0], xb_sb[:, 0], rs_sb[:])

            # phase 2: ACT exp h1; PE ctx h0 lagged one it behind the scores
            for it in range(IT):
                sc_exp(1, it)
                rinv_it(1, it)
                nc.vector.tensor_scalar_mul(
                    vt8_sb[:, 1, it], v1_sb[:, it], rinv[:, 1, ds(it, 1)]
                )
                if it in (2, 6, 10):
                    ct = it // 4 + 1
                    nc.vector.tensor_scalar_mul(outa[:, ct], xb_sb[:, ct], rs_sb[:])
                if it >= 1:
                    t = it - 1
                    ctx_chunk(0, t // JT, t % JT, dma_out=False)
            ctx_chunk(0, 3, 3, dma_out=False)

            # phase 3: ctx h1, ACT idle
            for et in range(ET):
                for jt in range(JT):
                    ctx_chunk(1, et, jt, dma_out=True)

    nc.finalize()
    return nc


def kernel(x, Wq, bq, Wk, bk, Wv, bv):
    global _NC_CACHE, LAST_EXEC_NS, LAST_MEAN_EXEC_NS
    x = np.ascontiguousarray(np.asarray(x, dtype=np.float32))
    Wq = np.asarray(Wq, dtype=np.float32)
    Wk = np.asarray(Wk, dtype=np.float32)
    Wv = np.asarray(Wv, dtype=np.float32)
    scale = np.float32(D ** -0.5)

    if _NC_CACHE is None:
        _NC_CACHE = _build()
    nc = _NC_CACHE

    x8 = x.astype(E4M3)
    xb = x.astype(ml_dtypes.bfloat16)

    # core c -> batch c//2, head pair c%2 (heads 2p, 2p+1)
    # wqs/wks: both heads of the pair stacked on the output axis -> [C, 128]
    wqs_pair = []
    wks_pair = []
    wv_pair = []
    for pair in range(2):
        hs = [2 * pair, 2 * pair + 1]
        wqs_pair.append(
            np.ascontiguousarray(
                np.concatenate(
                    [Wq[h].T * (QK_SCALE * scale) for h in hs], axis=1
                ).astype(E4M3)
            )
        )
        wks_pair.append(
            np.ascontiguousarray(
                np.concatenate([Wk[h].T * QK_SCALE for h in hs], axis=1).astype(E4M3)
            )
        )
        wv_pair.append(
            np.ascontiguousarray(
                (np.stack([Wv[h].T for h in hs]) * V_SCALE).astype(E4M3)
            )
        )

    in_maps = []
    for c in range(8):
        b, pair = c // 2, c % 2
        in_maps.append(
            {
                "x8": x8[b],
                "xb": xb[b],
                "wqs": wqs_pair[pair],
                "wks": wks_pair[pair],
                "wv": wv_pair[pair],
                "rs": np.full(
                    (P, 1), 2.0 * GAMMA if pair == 0 else 0.0, dtype=np.float32
                ),
            }
        )

    res = bass_utils.run_bass_kernel_spmd(nc, in_maps, core_ids=list(range(8)))
    LAST_EXEC_NS = res.exec_time_ns
    LAST_MEAN_EXEC_NS = res.mean_exec_time_ns

    out = np.empty((B, C, W), dtype=np.float32)
    inv_g = np.float32(1.0 / GAMMA)
    for b in range(B):
        out[b] = (res.results[2 * b]["out"] + res.results[2 * b + 1]["out"]) * inv_g
    return out
